# revision 16
# baseline (speedup 1.0000x reference)
"""2-layer GAT (100000 nodes, 32 neighbors) on 8 trn2 NeuronCores.

Strategy (SPMD, one Bass program for all 8 cores):
  - Nodes sharded 8 ways (12500/core); weights replicated and fused on the
    host ([W | W@A1blk | W@A2blk]) so one PE matmul per 128-node chunk
    emits h, s1 and s2 together. Everything on-chip is fp16 with h stored
    F-MAJOR (k innermost) so the big DVE tensor_tensor ops hit the packed
    2x mode.
  - Tables are BANKED: 4 banks of 25216 rows (25088 real + a sentinel row
    whose s2 = -6e4, so exp() zeroes padded slots). Bank-local indices fit
    int16, which unlocks dma_gather: ONE batched gather per (tile, bank)
    instead of 33 indirect DMAs per tile (the ~1us SWDGE fixed cost is
    amortized over ~2000 rows instead of 128).
  - Neighbor slots are bank-compacted per partition (attention is
    order-invariant over neighbor slots). Each bank block carries one extra
    SELF slot (the destination node's own row, sentinel in 3 of 4 banks):
    s1 = sum of the 4 self slots' s1 columns, and the self slots' s2 are
    then clamped to -6e4 so they stay out of the softmax.
  - The gather schedule (slots per bank per tile, maxed across cores) is
    baked into the program at build time; the kernel rebuilds if the
    neighbor table changes.
  - Layer-1 outputs x2 [64]f16 are PE-transposed, AllGathered as x2T
    (12.8 MB) and used to build the layer-2 table.
Output: per-core [12500,16]f32 shard, concatenated on the host.
"""
import sys

if '/opt/trn_rl_repo' not in sys.path:
    sys.path.insert(0, '/opt/trn_rl_repo')

import numpy as np
import concourse.bass as bass
import concourse.bacc as bacc
import concourse.mybir as mybir
from concourse.tile import TileContext
from concourse.masks import make_identity

import jax
from jax.sharding import Mesh, PartitionSpec
from jax.experimental.shard_map import shard_map
from concourse.bass2jax import (_bass_exec_p, install_neuronx_cc_hook,
                                partition_id_tensor)

FP = mybir.dt.float32
F16 = mybir.dt.float16
I16 = mybir.dt.int16
AF = mybir.ActivationFunctionType
OP = mybir.AluOpType
AX = mybir.AxisListType

N_NODES = 100000
N_CORES = 8
D_NBR = 32
K1, F1 = 8, 8
K2, F2 = 8, 16
H1, H2 = K1 * F1, K2 * F2            # 64, 128
R1, R2 = H1 + 16, H2 + 16            # used row elems: 80 / 144
NEG_SLOPE = 0.01
GT = 8            # chunks per table-build group
GA = 4            # tiles per attention group
BKS = 25088       # real rows per bank (multiple of 128)
REG = 25216       # bank region rows incl. sentinel (multiple of 128)
SENT = 25088      # sentinel local row id
NBANK = 4
TROWS = NBANK * REG
E1, E2 = 128, 256                    # table row elems (f16): 256B / 512B
SENT_NEG = -60000.0


def _plan_from_neighbors(neighbors):
    """Cross-core-max gather schedule + per-core packed int16 index data."""
    S = N_NODES // N_CORES
    n_tiles = (S + 127) // 128
    pad = n_tiles * 128 - S
    nbr = np.asarray(neighbors).astype(np.int64).reshape(N_CORES, S, D_NBR)
    bank = (nbr // BKS).astype(np.int32)
    local = (nbr % BKS).astype(np.int32)
    if pad:
        bank = np.concatenate(
            [bank, np.full((N_CORES, pad, D_NBR), -1, np.int32)], axis=1)
        local = np.concatenate(
            [local, np.zeros((N_CORES, pad, D_NBR), np.int32)], axis=1)
    bank = bank.reshape(N_CORES, n_tiles, 128, D_NBR)
    local = local.reshape(N_CORES, n_tiles, 128, D_NBR)

    cnt = np.stack([(bank == b).sum(axis=-1) for b in range(NBANK)], axis=-1)
    # pad partitions (tail tile) get one real request (bank0 row0) so their
    # softmax normalizer stays finite
    if pad:
        cnt[:, n_tiles - 1, 128 - pad:, 0] = np.maximum(
            cnt[:, n_tiles - 1, 128 - pad:, 0], 1)
    m = cnt.max(axis=(0, 2)).astype(np.int64)    # [n_tiles, NBANK]
    m = np.maximum(m, 1)
    # slots per tile = sum_b (m_b + 1 self); pad to a multiple of 4 for the
    # two tree-add levels before the slot reduction
    mt = m.sum(axis=1) + NBANK
    m[:, 0] += (-mt) % 4
    mt = m.sum(axis=1) + NBANK

    cols_per_tile = 8 * (m + 1).sum(axis=1)      # int16 cols per tile
    col_off = np.zeros(n_tiles + 1, np.int64)
    col_off[1:] = np.cumsum(cols_per_tile)
    total_cols = int(col_off[-1])

    gidx = np.full((N_CORES, 16, total_cols), SENT, np.int16)
    own_all = np.arange(N_CORES * S, dtype=np.int64)
    for r in range(N_CORES):
        for t in range(n_tiles):
            base = int(col_off[t])
            for b in range(NBANK):
                mb = int(m[t, b])
                L = 128 * (mb + 1)
                lst = np.full((mb + 1, 128), SENT, np.int32)
                for p in range(128):
                    node = r * S + t * 128 + p
                    sel = local[r, t, p][bank[r, t, p] == b]
                    lst[:len(sel), p] = sel
                    if b == 0 and len(sel) == 0 and t == n_tiles - 1 \
                            and p >= 128 - pad:
                        lst[0, p] = 0        # keep z finite on pad rows
                    if node < (r + 1) * S and node // BKS == b:
                        lst[mb, p] = node % BKS      # self slot
                flat = lst.reshape(-1)
                gidx[r, :, base:base + L // 16] = \
                    flat.reshape(L // 16, 16).T.astype(np.int16)
                base += L // 16
    gidx_rep = np.ascontiguousarray(np.tile(gidx, (1, 8, 1)))
    return {
        'm': m, 'mt': mt, 'col_off': col_off, 'total_cols': total_cols,
        'gidx': gidx_rep, 'n_tiles': n_tiles, 'S': S,
    }


def _build_gat(plan):
    S = plan['S']
    n_tiles = plan['n_tiles']
    m = plan['m']
    mt = plan['mt']
    col_off = plan['col_off']
    total_cols = plan['total_cols']
    n_cores = N_CORES
    N = N_NODES
    IN1 = 128

    nc = bacc.Bacc("TRN2", target_bir_lowering=False, debug=False,
                   num_devices=n_cores)
    xT = nc.dram_tensor("xT", [IN1, N], F16, kind="ExternalInput").ap()
    rhs1 = nc.dram_tensor("rhs1", [IN1, R1], F16, kind="ExternalInput").ap()
    rhs2 = nc.dram_tensor("rhs2", [H1, R2], F16, kind="ExternalInput").ap()
    gidx = nc.dram_tensor("gidx", [128, total_cols], I16,
                          kind="ExternalInput").ap()
    sent = nc.dram_tensor("sent", [2, E2], F16, kind="ExternalInput").ap()
    out = nc.dram_tensor("out", [S, F2], FP, kind="ExternalOutput").ap()

    table1 = nc.dram_tensor("table1", [TROWS, E1], F16).ap()
    table2 = nc.dram_tensor("table2", [TROWS, E2], F16).ap()
    x2T_shard = nc.dram_tensor("x2T_shard", [H1, S], F16).ap()
    x2T_all = nc.dram_tensor("x2T_all", [n_cores * H1, S], F16,
                             addr_space="Shared").ap()
    x2T_bounce = nc.dram_tensor("x2T_bounce", [H1, S], F16).ap()

    def g_of(n):
        return (n // BKS) * REG + (n % BKS)

    # table-build spans (original node space), split at bank boundaries so
    # each span is 128-aligned and affine after remapping
    full_chunks = N // 128
    tail_m = N - full_chunks * 128
    spans = []
    for c0 in range(0, full_chunks, GT):
        lo, hi = c0 * 128, min(c0 + GT, full_chunks) * 128
        cut = (lo // BKS + 1) * BKS
        if lo < cut < hi:
            spans.extend([(lo, cut), (cut, hi)])
        else:
            spans.append((lo, hi))

    tgroups = [(g, min(g + GA, n_tiles)) for g in range(0, n_tiles, GA)]

    with TileContext(nc) as tc:
        with tc.tile_pool(name="const", bufs=1) as cpool, \
             tc.tile_pool(name="tb", bufs=2) as tbp, \
             tc.tile_pool(name="att", bufs=2) as ap_, \
             tc.tile_pool(name="atth", bufs=3) as hp_, \
             tc.tile_pool(name="attt", bufs=1) as tp_, \
             tc.tile_pool(name="grp", bufs=2) as gp, \
             tc.tile_pool(name="psum", bufs=2, space="PSUM") as pp:

            rt1 = cpool.tile([IN1, R1], F16)
            nc.sync.dma_start(out=rt1[:], in_=rhs1[:, :])
            rt2 = cpool.tile([H1, R2], F16)
            nc.sync.dma_start(out=rt2[:], in_=rhs2[:, :])
            sct = cpool.tile([2, E2], F16)
            nc.sync.dma_start(out=sct[:], in_=sent[:, :])
            ident = cpool.tile([128, 128], F16)
            make_identity(nc, ident[:])

            for b in range(NBANK):
                srow = b * REG + SENT
                nc.sync.dma_start(out=table1[srow:srow + 1, :],
                                  in_=sct[0:1, :E1])
                nc.sync.dma_start(out=table2[srow:srow + 1, :],
                                  in_=sct[1:2, :E2])

            # ---- T-phase helper: one span of full 128-chunks ----
            def t_span(lo, hi, rt, used, table, lhs_rows, load_fn, tag):
                W = hi - lo
                nch = W // 128
                lt = tbp.tile([lhs_rows, GT * 128], F16,
                              name=f"{tag}l{lo}", tag=f"{tag}l")
                load_fn(lt, lo, W)
                rowt = tbp.tile([128, GT * used], F16, name=f"{tag}r{lo}",
                                tag=f"{tag}r")
                for ci in range(nch):
                    ps = pp.tile([128, used], FP, name=f"{tag}p{lo}_{ci}",
                                 tag=f"{tag}p", space="PSUM")
                    nc.tensor.matmul(out=ps[:],
                                     lhsT=lt[:, ci * 128:(ci + 1) * 128],
                                     rhs=rt[:], start=True, stop=True)
                    dst = rowt[:, ci * used:(ci + 1) * used]
                    if ci % 2 == 0:
                        nc.vector.tensor_copy(out=dst, in_=ps[:])
                    else:
                        nc.scalar.activation(out=dst, in_=ps[:], func=AF.Copy)
                gs = g_of(lo)
                nc.sync.dma_start(
                    out=table[gs:gs + W, :used].rearrange(
                        "(c p) r -> p c r", p=128),
                    in_=rowt[:].rearrange("p (c r) -> p c r", r=used)
                        [:, :nch, :])

            # ---- phase T1 ----
            def t1_load(lt, lo, W):
                nc.sync.dma_start(out=lt[:, :W], in_=xT[:, lo:lo + W])

            for (lo, hi) in spans:
                t_span(lo, hi, rt1, R1, table1, IN1, t1_load, "t1")
            if tail_m:
                lo = full_chunks * 128
                lt = tbp.tile([IN1, 128], F16, name="t1lz", tag="t1lz")
                nc.sync.dma_start(out=lt[:, :tail_m], in_=xT[:, lo:lo + tail_m])
                ps = pp.tile([128, R1], FP, name="t1pz", tag="t1p",
                             space="PSUM")
                nc.tensor.matmul(out=ps[:tail_m, :], lhsT=lt[:, :tail_m],
                                 rhs=rt1[:], start=True, stop=True)
                rowt = tbp.tile([128, R1], F16, name="t1rz", tag="t1rz")
                nc.vector.tensor_copy(out=rowt[:tail_m, :], in_=ps[:tail_m, :])
                gs = g_of(lo)
                nc.sync.dma_start(out=table1[gs:gs + tail_m, :R1],
                                  in_=rowt[:tail_m, :])

            # ---- attention for one tile (both layers share this shape) ----
            MT_MAX = int(mt.max())

            def attention(itile, icol0, t, table, elem, used, Kh, Fh, tagp,
                          emit):
                Mt = int(mt[t])
                hg = hp_.tile([128, MT_MAX * elem], F16, name=f"g{tagp}{t}",
                              tag="gA")
                hgv = hg[:, :Mt * elem].rearrange("p (s e) -> p s e", e=elem)
                soff, ioff = 0, icol0
                selfpos = []
                for b in range(NBANK):
                    mb = int(m[t, b])
                    # num_idxs > 1024 needs single_packet=False, which is
                    # flaky on this backend -> chunk into <=8-slot calls
                    done = 0
                    while done < mb + 1:
                        w = min(8, mb + 1 - done)
                        nc.gpsimd.dma_gather(
                            out_ap=hgv[:, soff + done:soff + done + w, :],
                            in_ap=table[b * REG:(b + 1) * REG, :],
                            idxs_ap=itile[:, ioff + 8 * done:
                                          ioff + 8 * (done + w)],
                            num_idxs=128 * w, num_idxs_reg=128 * w,
                            elem_size=elem)
                        done += w
                    selfpos.append(soff + mb)
                    soff += mb + 1
                    ioff += 8 * (mb + 1)
                # s1 = sum of the 4 self slots' s1 columns (sentinels are 0)
                s1a = ap_.tile([128, Kh], F16, name=f"sa{tagp}{t}",
                               tag="saA")
                nc.vector.tensor_tensor(
                    out=s1a[:], in0=hgv[:, selfpos[0], used:used + Kh],
                    in1=hgv[:, selfpos[1], used:used + Kh], op=OP.add)
                s1b = ap_.tile([128, Kh], F16, name=f"sb{tagp}{t}",
                               tag="sbA")
                nc.vector.tensor_tensor(
                    out=s1b[:], in0=hgv[:, selfpos[2], used:used + Kh],
                    in1=hgv[:, selfpos[3], used:used + Kh], op=OP.add)
                s1t = ap_.tile([128, Kh], F16, name=f"sc{tagp}{t}",
                               tag="scA")
                nc.vector.tensor_tensor(out=s1t[:], in0=s1a[:], in1=s1b[:],
                                        op=OP.add)
                # clamp self slots' s2 so they vanish from the softmax
                for sp in selfpos:
                    nc.vector.tensor_scalar_min(
                        out=hgv[:, sp, used + Kh:used + 2 * Kh],
                        in0=hgv[:, sp, used + Kh:used + 2 * Kh],
                        scalar1=SENT_NEG)
                e = ap_.tile([128, MT_MAX * Kh], F16, name=f"e{tagp}{t}",
                             tag="eA")
                nc.vector.tensor_tensor(
                    out=e[:, :Mt * Kh].rearrange("p (s k) -> p s k", k=Kh),
                    in0=hgv[:, :, used + Kh:used + 2 * Kh],
                    in1=s1t[:].unsqueeze(1).to_broadcast([128, Mt, Kh]),
                    op=OP.add)
                u = ap_.tile([128, MT_MAX * Kh], F16, name=f"u{tagp}{t}",
                             tag="uA")
                nc.scalar.activation(out=u[:, :Mt * Kh], in_=e[:, :Mt * Kh],
                                     func=AF.Lrelu, alpha=NEG_SLOPE)
                nc.scalar.activation(out=u[:, :Mt * Kh], in_=u[:, :Mt * Kh],
                                     func=AF.Exp)
                z = ap_.tile([128, Kh], FP, name=f"z{tagp}{t}", tag="zA")
                nc.vector.tensor_reduce(
                    out=z[:],
                    in_=u[:, :Mt * Kh].rearrange("p (s k) -> p k s", k=Kh),
                    axis=AX.X, op=OP.add)
                rz = ap_.tile([128, Kh], FP, name=f"r{tagp}{t}",
                              tag="rA")
                nc.vector.reciprocal(out=rz[:], in_=z[:])
                tmp = tp_.tile([128, Fh * MT_MAX * Kh], F16,
                               name=f"t{tagp}{t}", tag="tA")
                tmpv = tmp[:, :Fh * Mt * Kh].rearrange(
                    "p (f s k) -> p f s k", s=Mt, k=Kh)
                h4 = hgv[:, :, 0:used].rearrange("p s (f k) -> p f s k", k=Kh)
                u4 = u[:, :Mt * Kh].rearrange("p (s k) -> p s k", k=Kh) \
                    .unsqueeze(1).to_broadcast([128, Fh, Mt, Kh])
                nc.vector.tensor_tensor(out=tmpv, in0=h4, in1=u4, op=OP.mult)
                # two tree-add levels over slots, then reduce
                q2, q4 = Mt // 2, Mt // 4
                nc.vector.tensor_tensor(
                    out=tmpv[:, :, :q2, :], in0=tmpv[:, :, :q2, :],
                    in1=tmpv[:, :, q2:2 * q2, :], op=OP.add)
                nc.vector.tensor_tensor(
                    out=tmpv[:, :, :q4, :], in0=tmpv[:, :, :q4, :],
                    in1=tmpv[:, :, q4:2 * q4, :], op=OP.add)
                sred = ap_.tile([128, Fh * Kh], FP, name=f"s{tagp}{t}",
                                tag="sA")
                nc.vector.tensor_reduce(
                    out=sred[:].rearrange("p (f k) -> p f k", k=Kh),
                    in_=tmpv[:, :, :q4, :].transpose([0, 1, 3, 2]),
                    axis=AX.X, op=OP.add)
                o = ap_.tile([128, Fh * Kh], F16, name=f"o{tagp}{t}",
                             tag="oA")
                nc.vector.tensor_tensor(
                    out=o[:].rearrange("p (f k) -> p f k", k=Kh),
                    in0=sred[:].rearrange("p (f k) -> p f k", k=Kh),
                    in1=rz[:].unsqueeze(1).to_broadcast([128, Fh, Kh]),
                    op=OP.mult)
                emit(o, t)

            # ---- phase A1 ----
            def emit1(o, t, xtg, c):
                mn = ap_.tile([128, H1], F16, name=f"m1{t}", tag="m1")
                nc.vector.tensor_scalar_min(out=mn[:], in0=o[:], scalar1=0.0)
                nc.scalar.activation(out=mn[:], in_=mn[:], func=AF.Exp)
                x2 = ap_.tile([128, H1], F16, name=f"x1{t}", tag="x1")
                nc.vector.scalar_tensor_tensor(
                    out=x2[:], in0=mn[:], scalar=-1.0, in1=o[:],
                    op0=OP.add, op1=OP.max)
                pt = pp.tile([H1, 128], F16, name=f"pt{t}", tag="pt",
                             space="PSUM")
                nc.tensor.transpose(out=pt[:], in_=x2[:], identity=ident[:])
                nc.vector.tensor_copy(out=xtg[:, c * 128:(c + 1) * 128],
                                      in_=pt[:])

            GCOL_MAX = max(int(col_off[t1] - col_off[t0])
                           for (t0, t1) in tgroups)

            for gi, (t0, t1) in enumerate(tgroups):
                c0, c1 = int(col_off[t0]), int(col_off[t1])
                itg = gp.tile([128, GCOL_MAX], I16,
                              name=f"i1{gi}", tag="i1")
                nc.sync.dma_start(out=itg[:, :c1 - c0], in_=gidx[:, c0:c1])
                xtg = gp.tile([H1, GA * 128], F16, name=f"xg{gi}", tag="xg")
                for t in range(t0, t1):
                    attention(itg[:], int(col_off[t]) - c0, t, table1, E1,
                              H1, K1, F1, "1",
                              lambda o, tt: emit1(o, tt, xtg, tt - t0))
                g0 = t0 * 128
                W = min(t1 * 128, S) - g0
                nc.sync.dma_start(out=x2T_shard[:, g0:g0 + W],
                                  in_=xtg[:, :W])

            # ---- phase AG ----
            nc.sync.dma_start(out=x2T_bounce[:, :], in_=x2T_shard[:, :])
            nc.gpsimd.collective_compute(
                "AllGather", OP.bypass,
                replica_groups=[list(range(n_cores))],
                ins=[x2T_bounce.opt()], outs=[x2T_all.opt()])

            # ---- phase T2 ----
            def t2_load(lt2, lo, W):
                done = 0
                while done < W:
                    r_a, i_a = divmod(lo + done, S)
                    n1 = min(W - done, S - i_a)
                    nc.sync.dma_start(
                        out=lt2[:, done:done + n1],
                        in_=x2T_all[r_a * H1:(r_a + 1) * H1, i_a:i_a + n1])
                    done += n1

            for (lo, hi) in spans:
                t_span(lo, hi, rt2, R2, table2, H1, t2_load, "t2")
            if tail_m:
                lo = full_chunks * 128
                lt2 = tbp.tile([H1, 128], F16, name="t2lz", tag="t2lz")
                t2_load(lt2, lo, tail_m)
                ps2 = pp.tile([128, R2], FP, name="t2pz", tag="t2p",
                              space="PSUM")
                nc.tensor.matmul(out=ps2[:tail_m, :], lhsT=lt2[:, :tail_m],
                                 rhs=rt2[:], start=True, stop=True)
                row2 = tbp.tile([128, R2], F16, name="t2rz", tag="t2rz")
                nc.vector.tensor_copy(out=row2[:tail_m, :], in_=ps2[:tail_m, :])
                gs = g_of(lo)
                nc.sync.dma_start(out=table2[gs:gs + tail_m, :R2],
                                  in_=row2[:tail_m, :])

            # ---- phase A2 ----
            def emit2(o2, t, og, c):
                mo = ap_.tile([128, F2], FP, name=f"mo{t}", tag="mo")
                nc.vector.tensor_reduce(
                    out=mo[:],
                    in_=o2[:].rearrange("p (f k) -> p f k", k=K2),
                    axis=AX.X, op=OP.add)
                u3 = ap_.tile([128, F2], FP, name=f"u3{t}", tag="u3")
                z3 = ap_.tile([128, 1], FP, name=f"z3{t}", tag="z3")
                nc.scalar.activation(out=u3[:], in_=mo[:], func=AF.Exp,
                                     scale=1.0 / K2, accum_out=z3[:])
                rz3 = ap_.tile([128, 1], FP, name=f"r3{t}", tag="r3")
                nc.vector.reciprocal(out=rz3[:], in_=z3[:])
                nc.vector.tensor_tensor(
                    out=og[:, c * F2:(c + 1) * F2], in0=u3[:],
                    in1=rz3[:].to_broadcast([128, F2]), op=OP.mult)

            for gi, (t0, t1) in enumerate(tgroups):
                c0, c1 = int(col_off[t0]), int(col_off[t1])
                itg = gp.tile([128, GCOL_MAX], I16,
                              name=f"i2{gi}", tag="i2")
                nc.sync.dma_start(out=itg[:, :c1 - c0], in_=gidx[:, c0:c1])
                og = gp.tile([128, GA * F2], FP, name=f"og{gi}", tag="og")
                for t in range(t0, t1):
                    attention(itg[:], int(col_off[t]) - c0, t, table2, E2,
                              H2, K2, F2, "2",
                              lambda o2, tt: emit2(o2, tt, og, tt - t0))
                g0 = t0 * 128
                W = min(t1 * 128, S) - g0
                nfull = W // 128
                if nfull:
                    nc.sync.dma_start(
                        out=out[g0:g0 + nfull * 128, :].rearrange(
                            "(c p) f -> p c f", p=128),
                        in_=og[:].rearrange("p (c f) -> p c f", f=F2)
                            [:, :nfull, :])
                if W % 128:
                    M = W % 128
                    nc.sync.dma_start(
                        out=out[g0 + nfull * 128:g0 + W, :],
                        in_=og[:M, nfull * F2:(nfull + 1) * F2])

    nc.finalize()
    return nc


class _SpmdRunner:
    """jit-once SPMD executor over the 8 axon NeuronCores."""

    def __init__(self, nc, n_cores):
        install_neuronx_cc_hook()
        self.nc, self.n_cores = nc, n_cores
        partition_name = (nc.partition_id_tensor.name
                          if nc.partition_id_tensor else None)
        in_names, out_names, out_avals, zero_outs = [], [], [], []
        for alloc in nc.m.functions[0].allocations:
            if not isinstance(alloc, mybir.MemoryLocationSet):
                continue
            name = alloc.memorylocations[0].name
            if alloc.kind == "ExternalInput":
                if name != partition_name:
                    in_names.append(name)
            elif alloc.kind == "ExternalOutput":
                out_names.append(name)
                shape = tuple(alloc.tensor_shape)
                dtype = mybir.dt.np(alloc.dtype)
                out_avals.append(jax.core.ShapedArray(shape, dtype))
                zero_outs.append(np.zeros(shape, dtype))
        self.in_names, self.out_names = in_names, out_names
        self.out_avals, self.zero_outs = out_avals, zero_outs
        all_in_names = in_names + out_names
        if partition_name is not None:
            all_in_names.append(partition_name)

        def _body(*args):
            operands = list(args)
            if partition_name is not None:
                operands.append(partition_id_tensor())
            return tuple(_bass_exec_p.bind(
                *operands, out_avals=tuple(out_avals),
                in_names=tuple(all_in_names), out_names=tuple(out_names),
                lowering_input_output_aliases=(),
                sim_require_finite=True, sim_require_nnan=True, nc=nc))

        devices = jax.devices()[:n_cores]
        self.mesh = Mesh(np.asarray(devices), ("core",))
        n_params, n_outs = len(in_names), len(out_avals)
        in_specs = (PartitionSpec("core"),) * (n_params + n_outs)
        out_specs = (PartitionSpec("core"),) * n_outs
        self.fn = jax.jit(
            shard_map(_body, mesh=self.mesh, in_specs=in_specs,
                      out_specs=out_specs, check_rep=False),
            keep_unused=True)
        self.sharding = jax.sharding.NamedSharding(self.mesh,
                                                   PartitionSpec("core"))

    def run(self, in_maps):
        per_core = [[np.asarray(m[n]) for n in self.in_names] for m in in_maps]
        concat = [np.concatenate([per_core[c][i] for c in range(self.n_cores)],
                                 axis=0) for i in range(len(self.in_names))]
        zeros = [np.zeros((self.n_cores * z.shape[0], *z.shape[1:]), z.dtype)
                 for z in self.zero_outs]
        dev = [jax.device_put(a, self.sharding) for a in concat + zeros]
        outs = self.fn(*dev)
        jax.block_until_ready(outs)
        res = []
        for c in range(self.n_cores):
            res.append({name: np.asarray(outs[i]).reshape(
                self.n_cores, *self.out_avals[i].shape)[c]
                for i, name in enumerate(self.out_names)})
        return res


def _host_prep(plan, node_features, neighbors, W1, a1_1, a2_1, W2, a1_2,
               a2_2):
    def blk(a, k, f):
        A = np.zeros((k * f, k), np.float32)
        for kk in range(k):
            A[kk * f:(kk + 1) * f, kk] = a[kk]
        return A

    W1p = W1.reshape(128, K1, F1).transpose(0, 2, 1).reshape(128, K1 * F1)
    rhs1 = np.concatenate(
        [W1p, W1 @ blk(a1_1, K1, F1), W1 @ blk(a2_1, K1, F1)],
        axis=1).astype(np.float16)
    W2r = W2.reshape(K1, F1, K2 * F2).transpose(1, 0, 2) \
        .reshape(K1 * F1, K2 * F2)
    W2p = W2r.reshape(K1 * F1, K2, F2).transpose(0, 2, 1) \
        .reshape(K1 * F1, K2 * F2)
    rhs2 = np.concatenate(
        [W2p, W2r @ blk(a1_2, K2, F2), W2r @ blk(a2_2, K2, F2)],
        axis=1).astype(np.float16)
    xT = np.ascontiguousarray(node_features.T).astype(np.float16)

    sent = np.zeros((2, E2), np.float16)
    sent[0, R1 - 8:R1] = SENT_NEG          # table1 s2 cols 72:80
    sent[1, R2 - 8:R2] = SENT_NEG          # table2 s2 cols 136:144

    in_maps = []
    for r in range(N_CORES):
        in_maps.append({'xT': xT, 'rhs1': rhs1, 'rhs2': rhs2,
                        'gidx': plan['gidx'][r], 'sent': sent})
    return in_maps


_RUNNER = None
_PLAN = None
_PLAN_KEY = None


def _get_runner(neighbors):
    global _RUNNER, _PLAN, _PLAN_KEY
    nb = np.asarray(neighbors)
    key = hash(nb.tobytes())
    if _RUNNER is None or key != _PLAN_KEY:
        _PLAN = _plan_from_neighbors(nb)
        _RUNNER = _SpmdRunner(_build_gat(_PLAN), N_CORES)
        _PLAN_KEY = key
    return _RUNNER, _PLAN


def kernel(node_features, neighbors, W1, a1_1, a2_1, W2, a1_2, a2_2):
    node_features = np.asarray(node_features, dtype=np.float32)
    neighbors = np.asarray(neighbors)
    runner, plan = _get_runner(neighbors)
    in_maps = _host_prep(plan, node_features, neighbors,
                         np.asarray(W1, np.float32),
                         np.asarray(a1_1, np.float32),
                         np.asarray(a2_1, np.float32),
                         np.asarray(W2, np.float32),
                         np.asarray(a1_2, np.float32),
                         np.asarray(a2_2, np.float32))
    res = runner.run(in_maps)
    return np.concatenate([res[c]['out'] for c in range(N_CORES)], axis=0)
